# revision 15
# baseline (speedup 1.0000x reference)
"""CapsuleLayer dynamic-routing kernel for Trainium2 (8 NeuronCores).

Problem: inputs [B=32, I=2048, J=16], W [N=64, I=2048, D=32, J=16], routings=3.
  inputs_hat[b,n,i,d] = sum_j inputs[b,i,j] * W[n,i,d,j]
  3 rounds of routing (softmax over n, weighted sum over i, squash over d).

Strategy: shard the input-capsule axis I across the 8 cores (I_loc=256).
All matmuls single-product bf16 (harness gate is rel_err < 2e-2).  Free-dim
order is (d, n) everywhere so c-broadcast multiplies keep innermost step=1
(DVE 2x bf16 mode).  Host pre-arranges every SBUF layout so all DMAs are
contiguous 4KB+ descriptors.

Round 0 (c uniform): s0 = (1/N) sum_{ij} x W via K=128 fused matmuls,
4-way column-tiled into 4 replica strips of srep, collapsed by a selector
matmul.  Rounds 1-2, per group of 4 capsules i:
  PE 16-tile mains (row=free-quarter, col=capsule): H[(4i,32b),(32d,64n)] f32
  ACT: evacuate H psum -> one SBUF bf16 tile per group
  DVE: y = sum_d H*vb (bf16 2x mul + 5-level halving tree); b += y
  (batched per 4 groups): c = softmax_n(b)
  DVE: tmp2 = c*H (bf16 2x, c broadcast on outer d axis)
  PE: srep[strip g%4] += sel.T @ tmp2 (fold partitions+i, 4-strip concurrent)
Then collapse strips, AllReduce s in bf16 (128 KB), squash on-chip.
Host reassembles [B,D,N] -> [B,N,D].
"""

import sys

for p in ("/opt/trn_rl_repo",):
    if p not in sys.path:
        sys.path.insert(0, p)

import ml_dtypes
import numpy as np

import concourse.bacc as bacc
import concourse.mybir as mybir
import concourse.tile as tile
from concourse.bass_utils import run_bass_kernel_spmd

# problem constants (hardcoded per harness contract)
B, N, I, D, J = 32, 64, 2048, 32, 16
R = 3  # routings
CORES = 8
I_LOC = I // CORES  # 256
DN = D * N  # 2048
EPS = 1e-7

F32 = mybir.dt.float32
BF16 = mybir.dt.bfloat16
FX = mybir.AxisListType.X
ADD = mybir.AluOpType.add
ACT = mybir.ActivationFunctionType

G = I_LOC // 4  # 64 groups of 4 capsules per round
SBATCH = 4      # softmax batch (groups)
FBATCH = 2      # fold flush batch (groups)
HF = DN // 2


def _squash_build(nc, vbp, sp, kp, s4, eps_ap, out32=None):
    """s4: [128, 2048] bf16 (d,n)-order s replicated x4 on partition groups.
    Returns vb [128, 2048] bf16 = squash(s).  If out32 given ([32,2048] f32
    tile), also writes fp32 squash for rows 0-31 (the host output)."""
    sqf = kp.tile([128, DN], F32, tag="sq_sqf", bufs=1)
    nc.scalar.activation(sqf[:], s4[:], ACT.Square)
    # halving tree over outer d: flat halves coincide with d halves
    cur = sqf
    w = DN // 2
    while w >= N:
        nxt = kp.tile([128, w], F32, tag=f"sq_t{w}", bufs=1)
        nc.vector.tensor_add(nxt[:], cur[:, 0:w], cur[:, w:2 * w])
        cur = nxt
        w //= 2
    sq = cur  # [128, 64] f32 = sum_d s^2 per n
    t = sp.tile([128, N], F32, tag="sq_t", bufs=1)
    nc.scalar.activation(t[:], sq[:], ACT.Sqrt, bias=eps_ap)
    q1 = sp.tile([128, N], F32, tag="sq_q1", bufs=1)
    nc.scalar.activation(q1[:], sq[:], ACT.Identity, bias=1.0)
    den = sp.tile([128, N], F32, tag="sq_den", bufs=1)
    nc.vector.tensor_mul(den[:], q1[:], t[:])
    rs = sp.tile([128, N], F32, tag="sq_rs", bufs=1)
    nc.vector.reciprocal(rs[:], den[:])
    scale = sp.tile([128, N], F32, tag="sq_scale", bufs=1)
    nc.vector.tensor_mul(scale[:], sq[:], rs[:])
    vb = vbp.tile([128, DN], BF16, tag="sq_vb")
    nc.vector.tensor_mul(
        vb[:].rearrange("p (d n) -> p d n", n=N),
        s4[:].rearrange("p (d n) -> p d n", n=N),
        scale[:, None, :].broadcast_to([128, D, N]),
    )
    if out32 is not None:
        nc.vector.tensor_mul(
            out32[:].rearrange("p (d n) -> p d n", n=N),
            s4[0:32, :].rearrange("p (d n) -> p d n", n=N),
            scale[0:32, None, :].broadcast_to([32, D, N]),
        )
    return vb


def build_kernel():
    nc = bacc.Bacc("TRN2", target_bir_lowering=False, debug=False)

    # host-prearranged inputs (all DMAs contiguous per partition):
    # xk[p, k, b] = x[b, i, j] with (i,j) = 128k+p      -- round-0 stationaries
    # xq[32q+j, i, b] = x[b, i, j], replicated q=0..3   -- 16-tile stationaries
    # wt[(i j), (d n)] = W[n, i, d, j]                  -- round-0 moving
    # wtp[32q+j, g, c, f] = wt[(4g+c) 16+j, 512q+f]     -- strip-padded moving
    xk = nc.dram_tensor("xk", [128, I_LOC * J // 128, B], BF16,
                        kind="ExternalInput")
    xq = nc.dram_tensor("xq", [128, I_LOC, B], BF16, kind="ExternalInput")
    wt = nc.dram_tensor("wt", [I_LOC * J, DN], BF16, kind="ExternalInput")
    wtp = nc.dram_tensor("wtp", [128, G, 4, 512], BF16, kind="ExternalInput")
    out = nc.dram_tensor("out", [B, DN], F32, kind="ExternalOutput")

    # collective bounce buffers (one pair per round), bf16 payload
    s_in = [nc.dram_tensor(f"s_in{r}", [B, DN], BF16) for r in range(R)]
    s_out = [nc.dram_tensor(f"s_out{r}", [B, DN], BF16, addr_space="Shared")
             for r in range(R)]

    with tile.TileContext(nc) as tc:
        with (
            tc.tile_pool(name="persist", bufs=1) as pp,
            tc.tile_pool(name="wr0", bufs=3) as wr0p,   # r0 W: [128,2,2048]b
            tc.tile_pool(name="wg", bufs=5) as wgp,     # rounds W: [128,4,512]b
            tc.tile_pool(name="hs", bufs=12) as hsp,    # evac'd H [128,2048]b
            tc.tile_pool(name="vbp", bufs=1) as vbp,
            tc.tile_pool(name="work", bufs=3) as kp,    # tree/work tiles
            tc.tile_pool(name="t2", bufs=6) as t2p,     # tmp2 [128,2048]b
            tc.tile_pool(name="cst", bufs=2) as cstp,
            tc.tile_pool(name="small", bufs=4) as sp,
            tc.tile_pool(name="psH", bufs=2, space="PSUM") as psH,  # [128,1024]
            tc.tile_pool(name="psS", bufs=1, space="PSUM") as psS,  # [128,2048]
        ):
            # ---- resident tiles (contiguous one-shot DMAs) ----
            xsb = pp.tile([128, I_LOC * J // 128, B], BF16, tag="xsb")
            nc.sync.dma_start(xsb[:], xk[:])
            xa4 = pp.tile([128, I_LOC, B], BF16, tag="xa4")
            nc.sync.dma_start(xa4[:], xq[:])

            bstate = pp.tile([128, G, N], BF16, tag="bstate")
            nc.gpsimd.memset(bstate[:], 0.0)
            eps_t = pp.tile([128, 1], F32, tag="eps")
            nc.gpsimd.memset(eps_t[:], EPS)
            # selector[p, m] = 1.0 if p % 32 == m  (partition-group fold)
            sel_i = pp.tile([128, B], mybir.dt.int32, tag="sel_i")
            nc.gpsimd.iota(sel_i[:], [[1, B]], channel_multiplier=-1)
            nc.vector.tensor_scalar(sel_i[:], sel_i[:], 31, None,
                                    op0=mybir.AluOpType.bitwise_and)
            sel32 = pp.tile([128, B], F32, tag="sel32")
            nc.vector.tensor_scalar(sel32[:], sel_i[:], 0, None,
                                    op0=mybir.AluOpType.is_equal)
            selb = pp.tile([128, B], BF16, tag="selb")
            nc.vector.tensor_copy(selb[:], sel32[:])

            s4 = pp.tile([128, DN], BF16, tag="s4")
            srb = pp.tile([128, DN], BF16, tag="srb")
            s_locb = pp.tile([32, DN], BF16, tag="s_locb")
            out32 = pp.tile([32, DN], F32, tag="out32")

            def collapse_A(r, srep, scale):
                # evac 4-strip replica psum -> bf16, fold strips, AllReduce
                nc.scalar.copy(srb[:], srep[:])
                cp0 = psH.tile([128, HF], F32, tag="ph")
                cp1 = psH.tile([128, HF], F32, tag="ph")
                cps = (cp0, cp1)
                for q in range(4):
                    nc.tensor.matmul(
                        cps[q // 2][0:32, 512 * (q % 2):512 * (q % 2) + 512],
                        selb[:], srb[:, 512 * q:512 * q + 512],
                        start=True, stop=True,
                    )
                for h in range(2):
                    nc.scalar.mul(s_locb[:, HF * h:HF * (h + 1)],
                                  cps[h][0:32, :], scale)
                nc.sync.dma_start(s_in[r][:], s_locb[:])
                nc.gpsimd.collective_compute(
                    "AllReduce", ADD,
                    replica_groups=[list(range(CORES))],
                    ins=[s_in[r].ap().opt()], outs=[s_out[r].ap().opt()],
                )

            def collapse_B(r):
                for g4 in range(4):
                    nc.sync.dma_start(s4[32 * g4:32 * (g4 + 1), :],
                                      s_out[r][:])

            PF = 4  # cross-round mains prefetch depth (groups)
            wg_q = {}   # (r, g) -> W tile
            hs_pre = {}  # (r, g) -> evacuated H tile

            def fetch_wg(r, gg):
                if 0 <= gg < G and (r, gg) not in wg_q:
                    w_ = wgp.tile([128, 4, 512], BF16, tag="wg",
                                  name=f"wg_r{r}g{gg}")
                    nc.sync.dma_start(w_[:], wtp[:, gg, :, :])
                    wg_q[(r, gg)] = w_

            def front(r, g):
                # mains + evac for (r, g); H lands in hs_pre
                fetch_wg(r, g)
                wg = wg_q.pop((r, g))
                ph0 = psH.tile([128, HF], F32, tag="ph")
                ph1 = psH.tile([128, HF], F32, tag="ph")
                phs = (ph0, ph1)
                # 16-tile mains: row strip q = free-quarter, col strip c
                for q in range(4):
                    for c in range(4):
                        nc.tensor.matmul(
                            phs[q // 2][32 * c:32 * c + 32,
                                        512 * (q % 2):512 * (q % 2) + 512],
                            xa4[32 * q:32 * q + 16, 4 * g + c, :],
                            wg[32 * q:32 * q + 16, c, :],
                            start=True, stop=True,
                            tile_position=(32 * q, 32 * c),
                        )
                hs = hsp.tile([128, DN], BF16, tag="hs", name=f"hs_r{r}g{g}")
                nc.scalar.copy(hs[:, 0:HF], ph0[:])
                nc.scalar.copy(hs[:, HF:DN], ph1[:])
                hs_pre[(r, g)] = hs

            def prefetch_round(r):
                if r <= 2:
                    for gg in range(PF):
                        fetch_wg(r, gg)
                    for gg in range(PF):
                        front(r, gg)

            # ---------- round 0: c uniform -> s0 = (1/N) sum_i ihat ----------
            srep = psS.tile([128, DN], F32, tag="srep")
            n_chunks = I_LOC * J // 128  # 32
            for kb4 in range(n_chunks // 4):
                wsbs = []
                for half in range(2):
                    w_ = wr0p.tile([128, 2, DN], BF16, tag="wr0",
                                   name=f"wr0_{kb4}_{half}")
                    for c2 in range(2):
                        row = 512 * kb4 + 256 * half + 128 * c2
                        nc.sync.dma_start(w_[:, c2, :],
                                          wt[row:row + 128, :])
                    wsbs.append(w_)
                for q in range(4):
                    for cc in range(4):
                        k = 4 * kb4 + cc
                        nc.tensor.matmul(
                            srep[32 * cc:32 * cc + 32,
                                 512 * q:512 * q + 512],
                            xsb[:, k, :],
                            wsbs[cc // 2][:, cc % 2, 512 * q:512 * q + 512],
                            start=(k < 4), stop=(k >= n_chunks - 4),
                            tile_position=(0, 32 * cc),
                            skip_group_check=True,
                        )
            collapse_A(0, srep, 1.0 / N)
            prefetch_round(1)
            collapse_B(0)
            vb = _squash_build(nc, vbp, sp, kp, s4, eps_t[:])

            # ---------- rounds 1, 2 ----------
            for r in (1, 2):
                srep = psS.tile([128, DN], F32, tag="srep")
                pend_tmp2 = []  # (g, hs) awaiting softmax c
                pend_fold = []  # (g, tm) awaiting fold flush

                def flush_folds(last=False, _srep=srep):
                    batch = list(pend_fold)
                    del pend_fold[:]
                    for f in range(4):
                        for g0, tm in batch:
                            s0 = 32 * (g0 % 4)
                            nc.tensor.matmul(
                                _srep[s0:s0 + 32, 512 * f:512 * f + 512],
                                selb[:],
                                tm[:, 512 * f:512 * f + 512],
                                start=(g0 < 4),
                                stop=(g0 >= G - 4),
                                tile_position=(0, s0),
                                skip_group_check=True,
                            )

                ebatch = [None]

                def do_softmax_batch(gb0):
                    # c = softmax_n(b) for groups gb0..gb0+SBATCH-1
                    # (per-group exp already done into ebatch)
                    e = ebatch[0]
                    se = sp.tile([128, SBATCH], F32, tag="se")
                    nc.vector.tensor_reduce(se[:], e[:], axis=FX, op=ADD)
                    rcp = sp.tile([128, SBATCH], F32, tag="rcp")
                    nc.vector.reciprocal(rcp[:], se[:])
                    cst = cstp.tile([128, SBATCH, N], BF16, tag="cst")
                    nc.vector.tensor_mul(
                        cst[:], e[:],
                        rcp[:, :, None].broadcast_to([128, SBATCH, N]))
                    # tmp2 = c * H for the groups that waited on this batch
                    while pend_tmp2:
                        gg, hsx = pend_tmp2.pop(0)
                        gi = gg - gb0
                        tm = t2p.tile([128, DN], BF16, tag="tm2")
                        nc.vector.tensor_mul(
                            tm[:].rearrange("p (d n) -> p d n", n=N),
                            hsx[:].rearrange("p (d n) -> p d n", n=N),
                            cst[:, gi, None, :].broadcast_to([128, D, N]),
                        )
                        pend_fold.append((gg, tm))

                for g in range(G):
                    fetch_wg(r, g + PF)
                    if (r, g) in hs_pre:
                        hs = hs_pre.pop((r, g))
                    else:
                        front(r, g)
                        hs = hs_pre.pop((r, g))
                    # y = sum_d H*vb : bf16 2x mul + 5-level halving tree
                    ty = kp.tile([128, DN], BF16, tag="ty", bufs=2)
                    nc.vector.tensor_mul(ty[:], hs[:], vb[:])
                    m16 = kp.tile([128, 1024], BF16, tag="m16", bufs=2)
                    nc.vector.tensor_add(m16[:], ty[:, 0:1024], ty[:, 1024:DN])
                    m8 = kp.tile([128, 512], BF16, tag="m8", bufs=2)
                    nc.vector.tensor_add(m8[:], m16[:, 0:512], m16[:, 512:1024])
                    m4 = kp.tile([128, 256], BF16, tag="m4", bufs=2)
                    nc.vector.tensor_add(m4[:], m8[:, 0:256], m8[:, 256:512])
                    m2 = kp.tile([128, 128], BF16, tag="m2", bufs=2)
                    nc.vector.tensor_add(m2[:], m4[:, 0:128], m4[:, 128:256])
                    y = sp.tile([128, N], BF16, tag="y")
                    nc.vector.tensor_add(y[:], m2[:, 0:N], m2[:, N:128])
                    bsl = bstate[:, g, :]
                    nc.vector.tensor_add(bsl, bsl, y[:])
                    if g % SBATCH == 0:
                        ebatch[0] = sp.tile([128, SBATCH, N], BF16, tag="e",
                                            bufs=2, name=f"e_r{r}b{g}")
                    nc.scalar.activation(ebatch[0][:, g % SBATCH, :], bsl,
                                         ACT.Exp)
                    pend_tmp2.append((g, hs))
                    if (g + 1) % SBATCH == 0:
                        do_softmax_batch(g + 1 - SBATCH)
                    if len(pend_fold) >= FBATCH:
                        flush_folds()
                flush_folds(last=True)
                collapse_A(r, srep, 1.0)
                prefetch_round(r + 1)
                collapse_B(r)
                vb = _squash_build(nc, vbp, sp, kp, s4, eps_t[:],
                                   out32=(out32 if r == 2 else None))

            nc.sync.dma_start(out[:], out32[:])

    nc.compile()
    return nc


_NC_CACHE = {}


def _get_nc():
    if "nc" not in _NC_CACHE:
        _NC_CACHE["nc"] = build_kernel()
    return _NC_CACHE["nc"]


def _make_in_maps(inputs, W):
    inputs = np.ascontiguousarray(np.asarray(inputs, dtype=np.float32))
    W = np.ascontiguousarray(np.asarray(W, dtype=np.float32))
    assert inputs.shape == (B, I, J) and W.shape == (N, I, D, J)
    in_maps = []
    for c in range(CORES):
        sl = slice(c * I_LOC, (c + 1) * I_LOC)
        x_t = inputs[:, sl, :].transpose(1, 2, 0)  # [i, j, b]
        # xk[p, k, b]: (i,j) = 128k+p
        x_k = x_t.reshape(I_LOC * J, B).reshape(32, 128, B).transpose(1, 0, 2)
        # xq[32q+j, i, b], q-replicated with 16-row padding
        x_jib = x_t.transpose(1, 0, 2)  # [j, i, b]
        x_q = np.zeros((4, 32, I_LOC, B), dtype=np.float32)
        x_q[:, 0:16] = x_jib[None, :, :, :]
        x_q = x_q.reshape(128, I_LOC, B)
        # wt[(i j), (d n)] = W[n, i, d, j]
        w_t = W[:, sl, :, :].transpose(1, 3, 2, 0).reshape(I_LOC * J, DN)
        # wtp[32q+j, g, c, f] = wt[(4g+c)16+j, 512q+f]
        w_4 = w_t.reshape(G, 4, J, 4, 512)  # [g, c, j, q, f]
        w_p = np.zeros((4, 32, G, 4, 512), dtype=np.float32)
        w_p[:, 0:16] = w_4.transpose(3, 2, 0, 1, 4)
        w_p = w_p.reshape(128, G, 4, 512)
        bf = ml_dtypes.bfloat16
        in_maps.append({
            "xk": np.ascontiguousarray(x_k.astype(bf)),
            "xq": np.ascontiguousarray(x_q.astype(bf)),
            "wt": np.ascontiguousarray(w_t.astype(bf)),
            "wtp": np.ascontiguousarray(w_p.astype(bf)),
        })
    return in_maps


def _ensure_ntff_hook():
    """Register the axon NTFF profile hook if the image's antenv lacks it."""
    import types

    try:
        import antenv.axon_hooks  # noqa: F401
        return
    except ImportError:
        pass
    import antenv

    if "/root/.axon_site" not in sys.path:
        sys.path.insert(0, "/root/.axon_site")
    from trn_agent_boot.trn_boot import _ntff_profile_via_ctypes

    hook = {"h": _ntff_profile_via_ctypes("/opt/axon/libaxon_pjrt.so")}
    mod = types.ModuleType("antenv.axon_hooks")
    mod.get_axon_ntff_profile_hook = lambda: hook["h"]
    mod.set_axon_ntff_profile_hook = lambda h: hook.__setitem__("h", h)
    sys.modules["antenv.axon_hooks"] = mod
    antenv.axon_hooks = mod


def run(inputs, W, trace=False):
    nc = _get_nc()
    if trace:
        _ensure_ntff_hook()
        # zero-egress container: skip the artifact upload, keep files local
        import concourse.bass_utils as bu
        bu.upload_artifacts = lambda d: d
    res = run_bass_kernel_spmd(
        nc, _make_in_maps(inputs, W), core_ids=list(range(CORES)),
        trace=trace,
    )
    o = res.results[0]["out"].reshape(B, D, N)
    return np.ascontiguousarray(o.transpose(0, 2, 1)), res


def kernel(inputs, W, routings=R, **_unused):
    assert int(routings) == R
    out, _ = run(inputs, W, trace=False)
    return out
